# revision 1
# baseline (speedup 1.0000x reference)
# Binarized 3x3 conv (per-direction / population-parallel), Trainium2 Bass kernel.
#
# Reference math: bits {0,1} -> {-1,+1}; out = 4*xw - 2*sx - 2*sw + K.
# Identity used here:  out = 2*conv(x, W2) - T[cout]
#   where W2 = 2w - 1 (+-1, exact in bf16), T[cout] = sum_{cin,kh,kw} W2,
#   conv is a standard zero-padded 3x3 conv with x in {0,1}.
# Proof: 2*sum(x*(2w-1)) - sum(2w-1) = 4xw - 2sx - (2sw - K).
#
# Sharding: D=64 directions split 8 per core across 8 NeuronCores (pure
# population parallelism, no communication).
#
# Per-core pipeline (per direction d):
#   - w[d] [9,128,128] f32 -> SBUF [cin,9,cout]; W2 = 2w-1 (bf16) on ACT/DVE
#   - x[d] tiles [128pix,128cin] u8 -> PE-transpose -> zero-padded channel-major
#     image xpad [cin, 34, 34] bf16
#   - 9 taps: matmul acc[cout, 512] += W2[tap].T @ xpad[window], two 512-pixel
#     blocks, plus accT[cout,1] += W2[tap].T @ ones (T on already-loaded weights)
#   - epilogue (ACT): outT[cout,1024] fp16 = 2*acc - T   (|out|<=1152, exact)
#   - PE-transpose back to [pix, cout] f32, DMA out contiguously.

import numpy as np

import concourse.bass as bass
import concourse.mybir as mybir
import concourse.tile as tile
from concourse import bacc
from concourse import bass_utils
from concourse.masks import make_identity

N_CORES = 8
D, H, W, CIN, COUT = 64, 32, 32, 128, 128
DPC = D // N_CORES  # directions per core
NPIX = H * W  # 1024
NT = 8  # 128-pixel tiles per image

FP32 = mybir.dt.float32
BF16 = mybir.dt.bfloat16
FP16 = mybir.dt.float16
I8 = mybir.dt.int8


def _body(nc, tc, x_d, w_d, o_d):
    Act = mybir.ActivationFunctionType
    Alu = mybir.AluOpType
    with (
        tc.tile_pool(name="const", bufs=1) as constp,
        tc.tile_pool(name="wf", bufs=3) as wfp,
        tc.tile_pool(name="w2", bufs=3) as w2p,
        tc.tile_pool(name="xpad", bufs=3) as xpp,
        tc.tile_pool(name="oT", bufs=2) as oTp,
        tc.tile_pool(name="of", bufs=3) as ofp,
        tc.tile_pool(name="nT", bufs=2) as nTp,
        tc.tile_pool(name="psA", bufs=2, space="PSUM") as psA,
        tc.tile_pool(name="psT", bufs=1, space="PSUM") as psT,
        tc.tile_pool(name="psX", bufs=2, space="PSUM") as psX,
        tc.tile_pool(name="psO", bufs=1, space="PSUM") as psO,
    ):
        # Kick off the first two x loads before anything else on the serial
        # gpsimd queue — their DMA latency dominates pipeline fill.
        xraw = constp.tile([128, DPC, NT, CIN], I8)
        xr0 = x_d.rearrange("d (t p) c -> p d t c", p=128)
        nc.gpsimd.dma_start(xraw[:, 0, 0:4], xr0[:, 0, 0:4])
        nc.gpsimd.dma_start(xraw[:, 0, 4:8], xr0[:, 0, 4:8])
        nc.gpsimd.dma_start(xraw[:, 1], xr0[:, 1])

        id_bf16 = constp.tile([128, 128], BF16)
        make_identity(nc, id_bf16)
        id_f16 = constp.tile([128, 128], FP16)
        make_identity(nc, id_f16)
        ones = constp.tile([128, 1], BF16)
        nc.gpsimd.memset(ones, 1.0)
        negone = constp.tile([128, 1], FP32)
        nc.gpsimd.memset(negone, -1.0)

        # x loads: per-direction RAW int8 SWDGE DMAs (casting DMAs are slow
        # in the SDMA datapath and starve the w loads); the i8->bf16 cast
        # runs on ACT, which casts at ~1 elem/lane/cycle (4x DVE/gpsimd).
        xall = constp.tile([128, DPC, NT, CIN], BF16)
        xr = xr0
        # First three padded-image tiles zeroed on DVE (idle during startup);
        # gpsimd's serial queue then only carries descgens + later memsets.
        early_xpads = []
        for d in range(3):
            exp = xpp.tile([128, 34 * 34], BF16, tag="xpad", name=f"xpad{d}")
            nc.vector.memset(exp, 0.0)
            early_xpads.append(exp)
        for d in range(2, DPC):
            nc.gpsimd.dma_start(xraw[:, d], xr[:, d])

        def x_path(d):
            # i8->bf16 cast on ACT, then 4 PE transposes -> one PSUM bank ->
            # one wide strided copy into the zero-padded channel-major image.
            if d == 0:
                nc.scalar.copy(xall[:, d, 0:4], xraw[:, d, 0:4])
                nc.scalar.copy(xall[:, d, 4:8], xraw[:, d, 4:8])
            else:
                nc.scalar.copy(xall[:, d], xraw[:, d])
            if d < 3:
                xpad = early_xpads[d]
            else:
                xpad = xpp.tile([128, 34 * 34], BF16, tag="xpad", name=f"xpad{d}")
                nc.gpsimd.memset(xpad, 0.0)
            xpad3 = xpad.rearrange("p (r c) -> p r c", r=34)
            for g in range(2):
                px = psX.tile([128, 512], BF16, tag="trx", name=f"px{d}{g}")
                for k in range(4):
                    t = 4 * g + k
                    nc.tensor.transpose(
                        px[:, k * 128 : (k + 1) * 128], xall[:, d, t, :], id_bf16
                    )
                nc.vector.tensor_copy(
                    xpad3[:, 16 * g + 1 : 16 * g + 17, 1:33],
                    px.rearrange("p (r c) -> p r c", c=32),
                )
            return xpad3

        xpad3_next = x_path(0)

        for d in range(DPC):
            xpad3 = xpad3_next

            # --- weights (HWDGE, parallel descgen), then W2 = 2w - 1 ---
            wf = wfp.tile([128, 9, COUT], FP32)
            nc.scalar.dma_start(wf, w_d[d].rearrange("t c o -> c t o"))
            w2 = w2p.tile([128, 9, COUT], BF16)
            w2f = w2.rearrange("p t o -> p (t o)")
            wff = wf.rearrange("p t o -> p (t o)")
            # halves split across DVE and ACT
            nc.vector.tensor_scalar(
                w2f[:, 0:576], wff[:, 0:576], 2.0, -1.0, Alu.mult, Alu.add
            )
            nc.scalar.activation(
                w2f[:, 576:1152], wff[:, 576:1152], Act.Identity,
                bias=negone, scale=2.0,
            )

            # --- 9-tap conv accumulation ---
            # Matmuls ordered by dependency depth so the PE has work while
            # the xpad copies land: accT needs only w2; block0 taps i<2 need
            # only copy-group 0 (pad row 0 + rows 1..16); everything else
            # needs group 1. start/stop flags are per PSUM bank group.
            acc = psA.tile([128, NPIX], FP32)
            accT = psT.tile([128, 1], FP32)
            for t in range(9):
                nc.tensor.matmul(
                    accT, lhsT=w2[:, t, :], rhs=ones, start=(t == 0), stop=(t == 8)
                )
            b0 = [(i, j) for i in (0, 1) for j in range(3)] + [
                (2, j) for j in range(3)
            ]
            for n, (i, j) in enumerate(b0):
                nc.tensor.matmul(
                    acc[:, 0:512],
                    lhsT=w2[:, 3 * i + j, :],
                    rhs=xpad3[:, i : i + 16, j : j + 32],
                    start=(n == 0),
                    stop=(n == 8),
                )
            for t in range(9):
                i, j = divmod(t, 3)
                nc.tensor.matmul(
                    acc[:, 512:1024],
                    lhsT=w2[:, t, :],
                    rhs=xpad3[:, 16 + i : 32 + i, j : j + 32],
                    start=(t == 0),
                    stop=(t == 8),
                )

            # Emit the NEXT direction's x-path before this direction's
            # epilogue/out-path: DVE then prioritizes the input copies the
            # next conv is waiting on over output copies that have slack.
            if d + 1 < DPC:
                xpad3_next = x_path(d + 1)

            # --- epilogue: 2*acc - T, fp16 (exact: integers <= 1152) ---
            negT = nTp.tile([128, 1], FP32)
            nc.scalar.activation(negT, accT, Act.Copy, scale=-1.0)
            oT = oTp.tile([128, NPIX], FP16)
            nc.scalar.activation(
                oT[:, 0:512], acc[:, 0:512], Act.Identity, bias=negT, scale=2.0
            )
            nc.vector.tensor_scalar(
                oT[:, 512:1024], acc[:, 512:1024], 2.0, negT, Alu.mult, Alu.add
            )

            # --- transpose back to [pix, cout] and store ---
            # Last direction uses quarter-granularity to shorten the serial
            # drain tail (epilogue -> transpose -> copy -> store chain).
            of = ofp.tile([128, NT, COUT], FP32)
            ngrp, per = (4, 2) if d == DPC - 1 else (2, 4)
            for g in range(ngrp):
                po = psO.tile([128, per * 128], FP16, tag="tro")
                for k in range(per):
                    kk = per * g + k
                    nc.tensor.transpose(
                        po[:, k * 128 : (k + 1) * 128],
                        oT[:, kk * 128 : (kk + 1) * 128],
                        id_f16,
                    )
                nc.vector.tensor_copy(
                    of[:, per * g : per * g + per, :],
                    po.rearrange("p (k o) -> p k o", k=per),
                )
                nc.sync.dma_start(
                    o_d[d].rearrange("(k p) o -> p k o", p=128)[
                        :, per * g : per * g + per
                    ],
                    of[:, per * g : per * g + per, :],
                )


_NC_CACHE = None


def _get_nc():
    global _NC_CACHE
    if _NC_CACHE is None:
        nc = bacc.Bacc(
            "TRN2", target_bir_lowering=False, debug=False, num_devices=N_CORES
        )
        x_d = nc.dram_tensor(
            "x_s", [DPC, NPIX, CIN], I8, kind="ExternalInput"
        ).ap()
        w_d = nc.dram_tensor(
            "w_s", [DPC, 9, CIN, COUT], FP32, kind="ExternalInput"
        ).ap()
        o_d = nc.dram_tensor(
            "out_s", [DPC, NPIX, COUT], FP32, kind="ExternalOutput"
        ).ap()
        with tile.TileContext(nc) as tc:
            _body(nc, tc, x_d, w_d, o_d)
        nc.compile()
        _NC_CACHE = nc
    return _NC_CACHE


def _in_maps(x, w):
    xs = np.ascontiguousarray(x).view(np.int8).reshape(D, NPIX, CIN)
    ws = np.ascontiguousarray(w, dtype=np.float32).reshape(D, 9, CIN, COUT)
    return [
        {"x_s": xs[c * DPC : (c + 1) * DPC], "w_s": ws[c * DPC : (c + 1) * DPC]}
        for c in range(N_CORES)
    ]


def kernel(x, w, _trace=False):
    nc = _get_nc()
    res = bass_utils.run_bass_kernel_spmd(
        nc, _in_maps(x, w), core_ids=list(range(N_CORES)), trace=_trace
    )
    out = np.concatenate([r["out_s"] for r in res.results], axis=0)
    out = out.reshape(D, H, W, COUT)
    if _trace:
        return out, res
    return out

